# revision 10
# baseline (speedup 1.0000x reference)
"""Trainium2 Bass kernel for ComputeFlowFields (homography warp + alpha composite).

Sharding: 8 cores = 4 batches x 2 H-halves (192 rows each); every core holds all
D=32 planes, so the composite cumprod needs no cross-core communication.

Per-core layout: tiles of [128 partitions = (d*4 + r), 512 x] covering 4 image
rows x 32 planes. Per tile:
  la = Ln(1 - a)                      (ACT, free affine)
  S  = L_strict @ la                  (PE matmul: exclusive cumsum over d)
  t  = Exp(S)                         (ACT, PSUM src)  -> transmittance
  ww = ws + c_t                       (DVE tensor_scalar, per-partition bias)
  rw = recip_approx(ww)               (DVE custom op)
  w  = a * t ;  wr = w * rw           (DVE tensor_tensor)
  SC = lhsT_t @ wr                    (PE matmul M=16: contract over d, giving
                                       per-row slope/intercept coefficients)
Host epilogue: out = xs * S + C  (tiny; the x-linearity of the warp lets the
device reduce over d before the per-pixel grid is ever materialized).
"""

import sys

sys.path.insert(0, "/opt/trn_rl_repo")

import numpy as np

B, D, H, W = 4, 32, 384, 512
HB = H // 2            # rows per core
NT = HB // 4           # 48 four-row tiles per core
NPAIR = NT // 2
P = 128
HWC = (W - 1) / 2.0
HHC = (H - 1) / 2.0

_DD = np.arange(P) % 32   # plane index per partition   (p = r*32 + d)
_RR = np.arange(P) // 32  # row-within-tile per partition

_cache = {}


def _build():
    import concourse.bacc as bacc
    import concourse.mybir as mybir
    import concourse.tile as tile

    fp32 = mybir.dt.float32
    AF = mybir.ActivationFunctionType

    nc = bacc.Bacc(None, target_bir_lowering=False)
    alpha = nc.dram_tensor("alpha", [D, HB, W], fp32, kind="ExternalInput")
    lmat = nc.dram_tensor("lmat", [P, P], fp32, kind="ExternalInput")
    wsd = nc.dram_tensor("ws", [P, W], fp32, kind="ExternalInput")
    cbias = nc.dram_tensor("cbias", [P, NT], fp32, kind="ExternalInput")
    lout = nc.dram_tensor("lout", [P, NT * 16], fp32, kind="ExternalInput")
    outsc = nc.dram_tensor("outsc", [NT, 16, W], fp32, kind="ExternalOutput")

    with tile.TileContext(nc) as tc:
        with (
            tc.tile_pool(name="const", bufs=1) as cp,
            tc.tile_pool(name="a", bufs=4) as ap_,
            tc.tile_pool(name="la", bufs=3) as lap,
            tc.tile_pool(name="te", bufs=3) as tep,
            tc.tile_pool(name="ww", bufs=3) as wwp,
            tc.tile_pool(name="rw", bufs=3) as rwp,
            tc.tile_pool(name="w", bufs=3) as wp,
            tc.tile_pool(name="wr", bufs=3) as wrp,
            tc.tile_pool(name="sc", bufs=3) as scp,
            tc.tile_pool(name="psS", bufs=3, space="PSUM") as psS,
            tc.tile_pool(name="psO", bufs=2, space="PSUM") as psO,
        ):
            L_t = cp.tile([P, P], fp32)
            ws_t = cp.tile([P, W], fp32)
            cb_t = cp.tile([P, NT], fp32)
            lo_t = cp.tile([P, NT * 16], fp32)
            nc.sync.dma_start(L_t[:], lmat[:])
            nc.sync.dma_start(ws_t[:], wsd[:])
            nc.sync.dma_start(cb_t[:], cbias[:])
            nc.sync.dma_start(lo_t[:], lout[:])

            NQ = NT // 4
            for v in range(NQ):
                a4 = ap_.tile([P, 4 * W], fp32, tag="a")
                for g in range(4):
                    for r in range(4):
                        nc.sync.dma_start(
                            a4[32 * r : 32 * r + 32, g * W : (g + 1) * W],
                            alpha[:, 16 * v + 4 * g + r, :],
                        )

                la4 = lap.tile([P, 4 * W], fp32, tag="la")
                nc.scalar.activation(la4[:], a4[:], AF.Ln, bias=1.0, scale=-1.0)

                te4 = tep.tile([P, 4 * W], fp32, tag="te")
                for h in range(2):  # two [P, 2W] PSUM tiles per quad
                    s2 = psS.tile([P, 2 * W], fp32, tag="s")
                    for g2 in range(2):
                        g = 2 * h + g2
                        nc.tensor.matmul(
                            s2[:, g2 * W : (g2 + 1) * W],
                            L_t[:],
                            la4[:, g * W : (g + 1) * W],
                            start=True,
                            stop=True,
                        )
                    nc.scalar.activation(
                        te4[:, 2 * h * W : 2 * (h + 1) * W], s2[:], AF.Exp
                    )

                ww4 = wwp.tile([P, 4 * W], fp32, tag="ww")
                for g in range(4):
                    t = 4 * v + g
                    nc.vector.tensor_scalar_add(
                        ww4[:, g * W : (g + 1) * W], ws_t[:], cb_t[:, t : t + 1]
                    )
                rw4 = rwp.tile([P, 4 * W], fp32, tag="rw")
                nc.vector.reciprocal_approx_fast(rw4[:], ww4[:])

                w4 = wp.tile([P, 4 * W], fp32, tag="w")
                nc.vector.tensor_mul(w4[:], a4[:], te4[:])
                wr4 = wrp.tile([P, 4 * W], fp32, tag="wr")
                nc.vector.tensor_mul(wr4[:], w4[:], rw4[:])

                po = psO.tile([P, W], fp32, tag="po")
                for g in range(4):
                    t = 4 * v + g
                    nc.tensor.matmul(
                        po[32 * g : 32 * g + 16, :],
                        lo_t[:, 16 * t : 16 * t + 16],
                        wr4[:, g * W : (g + 1) * W],
                        start=True,
                        stop=True,
                        tile_position=(0, 32 * g),
                    )
                sc = scp.tile([P, W], fp32, tag="sc")
                nc.scalar.copy(sc[:], po[:])
                for g in range(4):
                    t = 4 * v + g
                    nc.sync.dma_start(outsc[t], sc[32 * g : 32 * g + 16, :])
    nc.compile()
    return nc


def _consts(h_mats):
    """Per-core host constants. Returns list of 8 dicts."""
    maps = []
    for c in range(8):
        b, hh = c // 2, c % 2
        hm = h_mats[b].astype(np.float64)  # [32,3,3]
        s_x = hm[:, 0, 0] / HWC - hm[:, 2, 0]
        t_x = hm[:, 0, 1] / HWC - hm[:, 2, 1]
        u_x = hm[:, 0, 2] / HWC - hm[:, 2, 2]
        s_y = hm[:, 1, 0] / HHC - hm[:, 2, 0]
        t_y = hm[:, 1, 1] / HHC - hm[:, 2, 1]
        u_y = hm[:, 1, 2] / HHC - hm[:, 2, 2]

        L = ((_RR[:, None] == _RR[None, :]) & (_DD[:, None] < _DD[None, :]))
        ws = hm[_DD, 2, 0][:, None] * np.arange(W)[None, :]
        tt = np.arange(NT)
        yy = 192 * hh + 4 * tt[None, :] + _RR[:, None]        # [P, NT]
        cb = hm[_DD, 2, 1][:, None] * yy + hm[_DD, 2, 2][:, None]

        lo = np.zeros((P, NT, 16))
        sv = np.stack([s_x, t_x, u_x, s_y, t_y, u_y])
        for m in range(16):
            q, ch, rm = m // 8, (m // 4) % 2, m % 4
            y = 192 * hh + 4 * tt + rm                         # [NT]
            if q == 0:
                val = np.broadcast_to(sv[3 * ch][_DD][:, None], (P, NT))
            else:
                val = sv[3 * ch + 1][_DD][:, None] * y[None, :] + sv[3 * ch + 2][_DD][:, None]
            lo[:, :, m] = (_RR == rm)[:, None] * val
        maps.append(
            {
                "lmat": L.astype(np.float32),
                "ws": np.ascontiguousarray(ws.astype(np.float32)),
                "cbias": np.ascontiguousarray(cb.astype(np.float32)),
                "lout": np.ascontiguousarray(lo.reshape(P, NT * 16).astype(np.float32)),
            }
        )
    return maps


def kernel(h_mats: np.ndarray, warped_alphas: np.ndarray) -> np.ndarray:
    from concourse.bass_utils import run_bass_kernel_spmd

    if "nc" not in _cache:
        _cache["nc"] = _build()
    nc = _cache["nc"]

    maps = _consts(np.asarray(h_mats))
    al = np.asarray(warped_alphas)
    for c in range(8):
        b, hh = c // 2, c % 2
        maps[c]["alpha"] = np.ascontiguousarray(
            al[b, :, 0, 192 * hh : 192 * hh + 192, :].astype(np.float32)
        )

    res = run_bass_kernel_spmd(nc, maps, core_ids=list(range(8)))
    out = np.zeros((B, H, W, 2), np.float32)
    xs = np.arange(W, dtype=np.float32)
    for c in range(8):
        b, hh = c // 2, c % 2
        SC = res.results[c]["outsc"]          # [NT, 16, W]
        gx = xs[None, None, :] * SC[:, 0:4] + SC[:, 8:12]
        gy = xs[None, None, :] * SC[:, 4:8] + SC[:, 12:16]
        out[b, 192 * hh : 192 * hh + 192, :, 0] = gx.reshape(HB, W)
        out[b, 192 * hh : 192 * hh + 192, :, 1] = gy.reshape(HB, W)
    return out
